# revision 7
# baseline (speedup 1.0000x reference)
# LoRA-MoE SwiGLU MLP kernel for 8 Trainium2 NeuronCores.
#
# Math (per token t, d_model D, mlp M, experts E, rank R):
#   logits = x @ Wr.T + br ; routing = softmax(logits) ; ec = onehot(argmax) (ST)
#   lora(x, A, B) = sum_e routing[:,e] * (x @ A.T) @ B_e.T * scaling
#   gate = x @ Wg.T + lora_g ; up = x @ Wu.T + lora_u
#   h = silu(gate) * up ; out = h @ Wd.T + lora_d(h)
#
# Strategy: data-parallel over the 4096 tokens (512/core); weights replicated.
# All heavy matmuls run as float32r (FP22 truncated fp32, 1 cycle/row on PE).
# Host pre-transposes weights/activations into contraction-major layouts so no
# on-chip weight transposes are needed. The E*R = 128 LoRA "experts x rank"
# axis is exactly one partition dim, so the combined per-expert LoRA matmul is
# a single K=128 matmul with the routing weights folded into the rhs.

import numpy as np

BB, SS, DD, MM, EE, RR = 4, 1024, 2048, 5632, 8, 16
SCALING = 2.0
N_CORES = 8


def build_nc(T, D, M, E, R):
    """Build the single-core Bass program (SPMD across cores)."""
    import concourse.bass as bass
    import concourse.bacc as bacc
    import concourse.mybir as mybir
    import concourse.tile as tile
    from concourse.masks import make_identity

    f32 = mybir.dt.float32
    f32r = mybir.dt.float32r
    P = 128
    KD = D // P           # d_model chunks
    KM = M // P           # mlp chunks
    TC = T // P           # token chunks
    ER = E * R            # must be 128
    assert ER == P

    nc = bacc.Bacc("TRN2", target_bir_lowering=False, debug=False)

    # ---- DRAM I/O ----
    xT = nc.dram_tensor("xT", [D, T], f32, kind="ExternalInput")
    WrT = nc.dram_tensor("WrT", [D, E], f32, kind="ExternalInput")
    br_rep = nc.dram_tensor("br_rep", [P, E], f32, kind="ExternalInput")
    AguT = nc.dram_tensor("AguT", [D, 2 * R], f32, kind="ExternalInput")
    AdT = nc.dram_tensor("AdT", [M, R], f32, kind="ExternalInput")
    WgT = nc.dram_tensor("WgT", [D, M], f32, kind="ExternalInput")
    WuT = nc.dram_tensor("WuT", [D, M], f32, kind="ExternalInput")
    WdT = nc.dram_tensor("WdT", [M, D], f32, kind="ExternalInput")
    BcatG = nc.dram_tensor("BcatG", [ER, M], f32, kind="ExternalInput")
    BcatU = nc.dram_tensor("BcatU", [ER, M], f32, kind="ExternalInput")
    BcatD = nc.dram_tensor("BcatD", [ER, D], f32, kind="ExternalInput")

    routing_o = nc.dram_tensor("routing_o", [T, E], f32, kind="ExternalOutput")
    ec_o = nc.dram_tensor("ec_o", [T, E], f32, kind="ExternalOutput")
    outT_o = nc.dram_tensor("outT_o", [D, T], f32, kind="ExternalOutput")

    AX = mybir.AxisListType
    OP = mybir.AluOpType
    ACT = mybir.ActivationFunctionType

    with tile.TileContext(nc) as tc:
        from contextlib import ExitStack
        es = ExitStack()
        const = es.enter_context(tc.tile_pool(name="const", bufs=1))
        sb_small = es.enter_context(tc.tile_pool(name="sb_small", bufs=3))
        sb_act = es.enter_context(tc.tile_pool(name="sb_act", bufs=3))
        ps_tp = es.enter_context(tc.tile_pool(name="ps_tp", bufs=2, space="PSUM"))

        # ---- resident SBUF tensors ----
        xt = const.tile([P, KD, T], f32r, name="xt")
        nc.sync.dma_start(xt, xT[:, :].rearrange("(k p) t -> p k t", p=P).bitcast(f32r))
        wr = const.tile([P, KD, E], f32r, name="wr")
        nc.sync.dma_start(wr, WrT[:, :].rearrange("(k p) e -> p k e", p=P).bitcast(f32r))
        brr = const.tile([P, E], f32, name="brr")
        nc.sync.dma_start(brr, br_rep[:, :])
        agu = const.tile([P, KD, 2 * R], f32r, name="agu")
        nc.sync.dma_start(agu, AguT[:, :].rearrange("(k p) r -> p k r", p=P).bitcast(f32r))
        adt = const.tile([P, KM, R], f32r, name="adt")
        nc.sync.dma_start(adt, AdT[:, :].rearrange("(k p) r -> p k r", p=P).bitcast(f32r))
        bcd = const.tile([P, D], f32r, name="bcd")
        nc.sync.dma_start(bcd, BcatD[:, :].bitcast(f32r))
        ident = const.tile([P, P], f32, name="ident")
        make_identity(nc, ident)

        routing_sb = const.tile([P, TC, E], f32, name="routing_sb")
        hwg = const.tile([P, T], f32r, name="hwg")
        hwu = const.tile([P, T], f32r, name="hwu")
        hwd = const.tile([P, T], f32r, name="hwd")
        h_sb = const.tile([P, KM, T], f32r, name="h_sb")

        # ---- phase 1: router softmax, expert-choice, LoRA hidden (gate/up) ----
        with tc.tile_pool(name="ps_small", bufs=2, space="PSUM") as ps_small:
            for c in range(TC):
                tsl = slice(c * P, (c + 1) * P)
                lg = ps_small.tile([P, E], f32, tag="lg")
                for k in range(KD):
                    nc.tensor.matmul(lg, xt[:, k, tsl], wr[:, k, :],
                                     start=(k == 0), stop=(k == KD - 1))
                lgs = sb_small.tile([P, E], f32, tag="lgs")
                nc.vector.tensor_add(lgs, lg, brr)
                negmax = sb_small.tile([P, 1], f32, tag="negmax")
                nc.vector.tensor_reduce(negmax, lgs, axis=AX.X, op=OP.max, negate=True)
                ex = sb_small.tile([P, E], f32, tag="ex")
                ssum = sb_small.tile([P, 1], f32, tag="ssum")
                nc.scalar.activation(ex, lgs, ACT.Exp, bias=negmax, accum_out=ssum)
                rs = sb_small.tile([P, 1], f32, tag="rs")
                nc.vector.reciprocal(rs, ssum)
                nc.vector.tensor_scalar_mul(routing_sb[:, c, :], ex, rs)
                nc.sync.dma_start(routing_o[tsl, :], routing_sb[:, c, :])

                # straight-through expert choice: (onehot - routing) + routing
                rmax = sb_small.tile([P, 1], f32, tag="rmax")
                nc.vector.tensor_reduce(rmax, routing_sb[:, c, :], axis=AX.X, op=OP.max)
                yh = sb_small.tile([P, E], f32, tag="yh")
                nc.vector.tensor_scalar(yh, routing_sb[:, c, :], rmax, None, op0=OP.is_equal)
                ecs = sb_small.tile([P, E], f32, tag="ecs")
                nc.vector.tensor_sub(ecs, yh, routing_sb[:, c, :])
                nc.vector.tensor_add(ecs, ecs, routing_sb[:, c, :])
                nc.sync.dma_start(ec_o[tsl, :], ecs)

                # LoRA hidden h' = x @ [Ag;Au].T  -> [tok, 2R]
                hp = ps_small.tile([P, 2 * R], f32, tag="hp")
                for k in range(KD):
                    nc.tensor.matmul(hp, xt[:, k, tsl], agu[:, k, :],
                                     start=(k == 0), stop=(k == KD - 1))
                hps = sb_small.tile([P, 2 * R], f32, tag="hps")
                nc.vector.tensor_copy(hps, hp)
                for gu, hw_sb in ((0, hwg), (1, hwu)):
                    hwp = sb_small.tile([P, E, R], f32, tag="hwp")
                    nc.vector.tensor_tensor(
                        hwp,
                        hps[:, None, gu * R:(gu + 1) * R].to_broadcast((P, E, R)),
                        routing_sb[:, c, :, None].to_broadcast((P, E, R)),
                        OP.mult)
                    tp = ps_tp.tile([P, P], f32, tag="tp")
                    nc.tensor.transpose(tp, hwp.rearrange("p e r -> p (e r)"), ident)
                    nc.vector.tensor_copy(hw_sb[:, tsl], tp)

        # ---- phase 2: gate/up big matmuls + LoRA + SwiGLU ----
        ps_mm = es.enter_context(tc.tile_pool(name="ps_mm", bufs=4, space="PSUM"))
        ps_hd = es.enter_context(tc.tile_pool(name="ps_hd", bufs=1, space="PSUM"))
        ph_d = ps_hd.tile([R, T], f32, name="ph_d")
        with tc.tile_pool(name="wgu", bufs=2) as wpool:
            for j in range(KM):
                msl = slice(j * P, (j + 1) * P)
                wg = wpool.tile([P, KD, P], f32r, tag="wg")
                nc.sync.dma_start(wg, WgT[:, :].rearrange("(k p) m -> p k m", p=P)[:, :, msl].bitcast(f32r))
                wu = wpool.tile([P, KD, P], f32r, tag="wu")
                nc.sync.dma_start(wu, WuT[:, :].rearrange("(k p) m -> p k m", p=P)[:, :, msl].bitcast(f32r))
                bg = wpool.tile([P, P], f32r, tag="bg")
                nc.sync.dma_start(bg, BcatG[:, msl].bitcast(f32r))
                bu = wpool.tile([P, P], f32r, tag="bu")
                nc.sync.dma_start(bu, BcatU[:, msl].bitcast(f32r))

                pg = ps_mm.tile([P, T], f32, tag="mm")
                for k in range(KD):
                    nc.tensor.matmul(pg, wg[:, k, :], xt[:, k, :],
                                     start=(k == 0), stop=False)
                nc.tensor.matmul(pg, bg, hwg, start=False, stop=True)
                pu = ps_mm.tile([P, T], f32, tag="mm")
                for k in range(KD):
                    nc.tensor.matmul(pu, wu[:, k, :], xt[:, k, :],
                                     start=(k == 0), stop=False)
                nc.tensor.matmul(pu, bu, hwu, start=False, stop=True)

                sg = sb_act.tile([P, T], f32, tag="sg")
                nc.scalar.activation(sg, pg, ACT.Sigmoid)
                nc.vector.tensor_mul(sg, sg, pg)
                nc.vector.tensor_mul(h_sb[:, j, :], sg, pu)

                # h_d accumulation, 2 chunks behind to keep PE dense
                if j >= 2:
                    nc.tensor.matmul(ph_d, adt[:, j - 2, :], h_sb[:, j - 2, :],
                                     start=(j - 2 == 0), stop=False)
            for jj in (KM - 2, KM - 1):
                nc.tensor.matmul(ph_d, adt[:, jj, :], h_sb[:, jj, :],
                                 start=(jj == 0), stop=(jj == KM - 1))

        # ---- phase 3: routing-weighted LoRA hidden for down proj ----
        hds = sb_small.tile([R, T], f32, name="hds")
        nc.vector.tensor_copy(hds, ph_d)
        for c in range(TC):
            tsl = slice(c * P, (c + 1) * P)
            tp1 = ps_tp.tile([P, P], f32, tag="tp")
            nc.tensor.transpose(tp1[:, :R], hds[:, tsl], ident[:R, :R])
            hdt = sb_small.tile([P, R], f32, tag="hdt")
            nc.vector.tensor_copy(hdt, tp1[:, :R])
            hwp = sb_small.tile([P, E, R], f32, tag="hwp")
            nc.vector.tensor_tensor(
                hwp,
                hdt[:, None, :].to_broadcast((P, E, R)),
                routing_sb[:, c, :, None].to_broadcast((P, E, R)),
                OP.mult)
            tp2 = ps_tp.tile([P, P], f32, tag="tp")
            nc.tensor.transpose(tp2, hwp.rearrange("p e r -> p (e r)"), ident)
            nc.vector.tensor_copy(hwd[:, tsl], tp2)

        # ---- phase 4: down proj + LoRA ----
        with tc.tile_pool(name="wd", bufs=2) as wdpool:
            for i in range(KD):
                dsl = slice(i * P, (i + 1) * P)
                wd = wdpool.tile([P, KM, P], f32r, tag="wd")
                nc.sync.dma_start(wd, WdT[:, :].rearrange("(k p) d -> p k d", p=P)[:, :, dsl].bitcast(f32r))
                pd = ps_mm.tile([P, T], f32, tag="mm")
                for k in range(KM):
                    nc.tensor.matmul(pd, wd[:, k, :], h_sb[:, k, :],
                                     start=(k == 0), stop=False)
                nc.tensor.matmul(pd, bcd[:, dsl], hwd, start=False, stop=True)
                od = sb_act.tile([P, T], f32, tag="od")
                nc.scalar.copy(od, pd)
                nc.sync.dma_start(
                    outT_o[:, :].rearrange("(k p) t -> p k t", p=P)[:, i, :], od)

        es.close()

    nc.compile()
    return nc


def _host_prep(x, Wr, br, Wg, Wu, Wd, Ag, Au, Ad, Bg, Bu, Bd, n_cores, scaling):
    """Shard + pre-transpose inputs into the kernel's layouts."""
    f = np.float32
    B, S, D = x.shape
    E, M, R = Bg.shape
    T = B * S // n_cores
    xf = np.ascontiguousarray(x.reshape(B * S, D))
    WrT = np.ascontiguousarray(Wr.T, dtype=f)
    br_rep = np.ascontiguousarray(np.tile(br[None, :], (128, 1)), dtype=f)
    AguT = np.ascontiguousarray(np.concatenate([Ag, Au], axis=0).T, dtype=f)
    AdT = np.ascontiguousarray(Ad.T, dtype=f)
    WgT = np.ascontiguousarray(Wg.T, dtype=f)
    WuT = np.ascontiguousarray(Wu.T, dtype=f)
    WdT = np.ascontiguousarray(Wd.T, dtype=f)
    BcatG = np.ascontiguousarray(
        (scaling * Bg).transpose(0, 2, 1).reshape(E * R, M), dtype=f)
    BcatU = np.ascontiguousarray(
        (scaling * Bu).transpose(0, 2, 1).reshape(E * R, M), dtype=f)
    BcatD = np.ascontiguousarray(
        (scaling * Bd).transpose(0, 2, 1).reshape(E * R, D), dtype=f)
    in_maps = []
    for c in range(n_cores):
        xTc = np.ascontiguousarray(xf[c * T:(c + 1) * T, :].T, dtype=f)
        in_maps.append({
            "xT": xTc, "WrT": WrT, "br_rep": br_rep, "AguT": AguT, "AdT": AdT,
            "WgT": WgT, "WuT": WuT, "WdT": WdT,
            "BcatG": BcatG, "BcatU": BcatU, "BcatD": BcatD,
        })
    return in_maps


_NC_CACHE = {}


def _get_nc(T, D, M, E, R):
    key = (T, D, M, E, R)
    if key not in _NC_CACHE:
        _NC_CACHE[key] = build_nc(T, D, M, E, R)
    return _NC_CACHE[key]


def kernel(x, Wr, br, Wg, Wu, Wd, Ag, Au, Ad, Bg, Bu, Bd, trace=False):
    from concourse.bass_utils import run_bass_kernel_spmd

    B, S, D = x.shape
    E, M, R = Bg.shape
    T = B * S // N_CORES
    nc = _get_nc(T, D, M, E, R)
    in_maps = _host_prep(x, Wr, br, Wg, Wu, Wd, Ag, Au, Ad, Bg, Bu, Bd,
                         N_CORES, SCALING)
    res = run_bass_kernel_spmd(nc, in_maps, core_ids=list(range(N_CORES)),
                               trace=trace)
    routing = np.concatenate([r["routing_o"] for r in res.results], axis=0)
    ec = np.concatenate([r["ec_o"] for r in res.results], axis=0)
    out = np.concatenate([r["outT_o"].T for r in res.results], axis=0)
    out = np.ascontiguousarray(out).reshape(B, S, D)
    routing = routing.reshape(B, S, E)
    ec = ec.reshape(B, S, E)
    if trace:
        kernel.last_results = res
    return out, routing, ec
